# revision 13
# baseline (speedup 1.0000x reference)
"""Trainium2 Bass kernel for nn_Custom_trainer_79242146611896.

Data-parallel over N=16384 samples on 8 NeuronCores (2048/core).
Per-class segment sums ([C,D]+counts) AllReduce'd early (overlapped with
the decode/reconstruction pass); the 3 scalar loss partials AllReduce'd
late (overlapped with the wgss tail). Weights replicated.

Speed strategy:
  * big matmuls in fp8e4m3 DoubleRow (K=256 per instruction, 0.5
    cycles/row); weights pre-scaled into fp8 normal range and descaled
    in the consuming activation.
  * DR matmuls are ordered for stationary-weight reuse (Ldweights is
    the hidden cost of DR): mm1 runs as k-sweeps over all sample
    chunks per (k,j) stationary; mm2 shares its encT stationaries with
    mm3; rec_latents runs over i-groups of 4 tiles per W2 stationary.
  * rec_latents = tanh(decoded @ W_enc) reuses W2 = W_dec @ W_enc.
  * decoded in natural [n,T] layout; rec pinball = fused
    scalar_tensor_tensor + abs-reduce, split DVE/ACT.
  * CCE exploits cat_labels == one_hot(labels).
  * gpsimd queue carries only weight casts + the two collectives so
    AllReduces never block compute.
"""

import numpy as np

import concourse.bass as bass
import concourse.mybir as mybir
import concourse.tile as tile
from concourse import bacc
from concourse.bass_utils import run_bass_kernel_spmd
from concourse.masks import make_identity

F32 = mybir.dt.float32
F32R = mybir.dt.float32r
BF16 = mybir.dt.bfloat16
F8 = mybir.dt.float8e4
I32 = mybir.dt.int32
AX = mybir.AxisListType
ALU = mybir.AluOpType
ACTF = mybir.ActivationFunctionType
DR = mybir.MatmulPerfMode.DoubleRow

P = 128
NCORES = 8
N_GLOBAL = 16384
T = 2048
D = 512
C = 50
KEPS = 1e-7

SW_ENC = 64.0
SW_DEC = 32.0
SW_CLS = 32.0
SW_W2 = 64.0
SW_MNS = 32.0

C1 = -float(np.log(KEPS))
C2 = -float(np.log1p(-KEPS))


def build(nl=N_GLOBAL // NCORES, n_global=None):
    n_global = n_global or NCORES * nl
    NT = T // P            # 16 K-tiles along T
    ND = D // P            # 4 K-tiles along D
    NN = nl // P           # 16 n-tiles per core
    NC = 256               # transpose chunk (samples)
    NCH = nl // NC         # 8 chunks
    NSUB = NC // P         # 2 n-subtiles per chunk

    nc = bacc.Bacc("TRN2", target_bir_lowering=False, debug=False, num_devices=NCORES)

    x_d = nc.dram_tensor("x", [nl, T], F32, kind="ExternalInput")
    o_d = nc.dram_tensor("output", [nl, T], F32, kind="ExternalInput")
    cl_d = nc.dram_tensor("cat_labels", [nl, C], F32, kind="ExternalInput")
    lab_d = nc.dram_tensor("labels", [nl], I32, kind="ExternalInput")
    wenc_d = nc.dram_tensor("W_enc", [T, D], F32, kind="ExternalInput")
    benc_d = nc.dram_tensor("b_enc", [D], F32, kind="ExternalInput")
    wdec_d = nc.dram_tensor("W_dec", [D, T], F32, kind="ExternalInput")
    bdec_d = nc.dram_tensor("b_dec", [T], F32, kind="ExternalInput")
    wcls_d = nc.dram_tensor("W_cls", [D, C], F32, kind="ExternalInput")
    bcls_d = nc.dram_tensor("b_cls", [C], F32, kind="ExternalInput")
    out_d = nc.dram_tensor("out", [nl], F32, kind="ExternalOutput")

    from contextlib import ExitStack

    with tile.TileContext(nc) as tc:
        with ExitStack() as ctx:
            ent = ctx.enter_context
            constp = ent(tc.tile_pool(name="const", bufs=1))
            wstg = ent(tc.tile_pool(name="wstg", bufs=1))
            wts = ent(tc.tile_pool(name="wts", bufs=1))
            encp = ent(tc.tile_pool(name="enc", bufs=1))
            accp = ent(tc.tile_pool(name="acc", bufs=1))
            xrowp = ent(tc.tile_pool(name="xrow", bufs=3))
            dtlp = ent(tc.tile_pool(name="dtl", bufs=3))
            ltp = ent(tc.tile_pool(name="lt", bufs=2))
            smallp = ent(tc.tile_pool(name="small", bufs=4))
            colsp = ent(tc.tile_pool(name="cols", bufs=4))
            junkp = ent(tc.tile_pool(name="junk", bufs=1))
            psm = ent(tc.tile_pool(name="psm", bufs=4, space="PSUM"))
            pssm = ent(tc.tile_pool(name="pssm", bufs=2, space="PSUM"))
            psper = ent(tc.tile_pool(name="psper", bufs=1, space="PSUM"))
            dp = ent(tc.tile_pool(name="dram", bufs=1, space="DRAM"))

            # ---------------- constants ----------------
            ident_f32 = constp.tile([P, P], F32)
            make_identity(nc, ident_f32)
            ident_f8 = constp.tile([P, P], F8)
            nc.vector.tensor_copy(ident_f8[:], ident_f32[:])

            ones_col = constp.tile([P, 1], F32)
            nc.any.memset(ones_col[:], 1.0)
            ones_col_f8 = constp.tile([P, 1], F8)
            nc.any.memset(ones_col_f8[:], 1.0)
            ones_k1f = constp.tile([1, P], F32)
            nc.any.memset(ones_k1f[:], 1.0)
            ones_pair = constp.tile([1, 2, P], F8)
            nc.any.memset(ones_pair[:, 0:1, :], 1.0)
            nc.any.memset(ones_pair[:, 1:2, :], 0.0)

            iot = constp.tile([P, C], I32)
            nc.gpsimd.iota(iot[:], [[1, C]], channel_multiplier=0)
            iotaf = constp.tile([P, C], F32)
            nc.vector.tensor_copy(iotaf[:], iot[:])

            # ---------------- weights: DMA + fp8 casts ----------------
            wenc_st = wstg.tile([P, NT, D], F32, tag="wstg")
            nc.sync.dma_start(
                wenc_st[:], wenc_d.ap().rearrange("(a p) d -> p a d", p=P)
            )
            wenc_f8 = wts.tile([P, NT, D], F8)
            for q in range(4):
                sl = (slice(None), slice(4 * q, 4 * q + 4), slice(None))
                nc.gpsimd.tensor_scalar(
                    out=wenc_f8[sl], in0=wenc_st[sl], scalar1=SW_ENC,
                    scalar2=None, op0=ALU.mult,
                )
            wdec_st = wstg.tile([P, ND, T], F32, tag="wstg")
            nc.sync.dma_start(
                wdec_st[:], wdec_d.ap().rearrange("(j p) t -> p j t", p=P)
            )
            wdec_f8 = wts.tile([P, ND, T], F8)
            for q in range(4):
                sl = (slice(None), slice(q, q + 1), slice(None))
                nc.gpsimd.tensor_scalar(
                    out=wdec_f8[sl], in0=wdec_st[sl], scalar1=SW_DEC,
                    scalar2=None, op0=ALU.mult,
                )
            wcls_st = wstg.tile([P, ND, C], F32, tag="wstg_cls")
            nc.sync.dma_start(
                wcls_st[:], wcls_d.ap().rearrange("(j p) c -> p j c", p=P)
            )
            wcls_f8 = wts.tile([P, ND, C], F8)
            nc.gpsimd.tensor_scalar(
                out=wcls_f8[:], in0=wcls_st[:], scalar1=SW_CLS,
                scalar2=None, op0=ALU.mult,
            )

            benc_r = benc_d.ap().rearrange("(k p) -> k p", p=P)
            bencT = []
            for k in range(ND):
                b_ = wts.tile([P, 1], F32, tag=f"bencT{k}")
                nc.sync.dma_start(b_[:], benc_r[k].rearrange("(p o) -> p o", o=1))
                bencT.append(b_)
            benc_row = wts.tile([1, D], F32)
            nc.sync.dma_start(
                benc_row[:], benc_d.ap().rearrange("(o d) -> o d", o=1)
            )
            bdec_row = wstg.tile([1, T], F32, tag="wstg_bdec")
            nc.sync.dma_start(
                bdec_row[:], bdec_d.ap().rearrange("(o t) -> o t", o=1)
            )
            bdec_pair = wts.tile([1, 2, T], F8)
            nc.any.memset(bdec_pair[:, 1:2, :], 0.0)
            nc.vector.tensor_scalar(
                out=bdec_pair[:, 0, :], in0=bdec_row[:], scalar1=SW_DEC,
                scalar2=None, op0=ALU.mult,
            )
            bdecT_f8 = wts.tile([P, NT, 1], F8)
            bdec_cr = bdec_d.ap().rearrange("(a p) -> a p", p=P)
            for a in range(NT):
                bc = colsp.tile([P, 1], F32, tag="bdec_cst")
                nc.sync.dma_start(bc[:], bdec_cr[a].rearrange("(p o) -> p o", o=1))
                nc.vector.tensor_scalar(
                    out=bdecT_f8[:, a, :], in0=bc[:], scalar1=SW_DEC,
                    scalar2=None, op0=ALU.mult,
                )
            bcls_row = wstg.tile([1, C], F32, tag="wstg_bcls")
            nc.sync.dma_start(
                bcls_row[:], bcls_d.ap().rearrange("(o c) -> o c", o=1)
            )
            bcls_pair = wts.tile([1, 2, C], F8)
            nc.any.memset(bcls_pair[:, 1:2, :], 0.0)
            nc.vector.tensor_scalar(
                out=bcls_pair[:, 0, :], in0=bcls_row[:], scalar1=SW_CLS,
                scalar2=None, op0=ALU.mult,
            )

            # ---------------- persistent activations ----------------
            xt_all = encp.tile([P, NT, nl], F8)   # [p, a, n] = x[n, a*128+p]
            encT = encp.tile([P, ND, nl], F8)     # [p, k, n] = enc[n, k*128+p]
            enc_nat = [
                encp.tile([P, D], F8, name=f"encnat{i2}", tag=f"encnat{i2}")
                for i2 in range(NN)
            ]
            onehot = [
                accp.tile([P, C], F8, name=f"oh{i}", tag=f"oh{i}") for i in range(NN)
            ]
            nsq_strip = accp.tile([P, NN], F32)
            rec_strip = accp.tile([P, 4 * NN], F32)
            lat_strip = accp.tile([P, 4 * ND], F32)
            cce_strip = accp.tile([P, NN], F32)
            gq_strip = accp.tile([P, NN], F32)
            junk_a = junkp.tile([P, D], BF16, tag="junk_a")
            junk_b = junkp.tile([P, D], BF16, tag="junk_b")
            junk_c = junkp.tile([P, C], BF16, tag="junk_c")

            seg_ps = psper.tile([C, D], F32)
            cnt_ps = psper.tile([C, 1], F32, tag="cnt")

            # ====== PASS 1a: stream x, PE-transpose into xt_all (fp8) ======
            for c in range(NCH):
                base = c * NC
                xr = []
                for s in range(NSUB):
                    r_ = xrowp.tile([P, T], F32, tag="xrow")
                    nc.sync.dma_start(
                        r_[:], x_d[base + s * P : base + (s + 1) * P, :]
                    )
                    xr.append(r_)
                cp_idx = 0
                for s in range(NSUB):
                    for a4 in range(NT // 4):
                        px = psm.tile([P, 4 * P], F32, tag="psm")
                        for da in range(4):
                            a = 4 * a4 + da
                            nc.tensor.transpose(
                                px[:, da * P : (da + 1) * P],
                                xr[s][:, a * P : (a + 1) * P],
                                ident_f32[:],
                            )
                        dst = xt_all[:, 4 * a4 : 4 * a4 + 4,
                                     base + s * P : base + (s + 1) * P]
                        src = px[:].rearrange("p (a n) -> p a n", a=4)
                        eng = (nc.scalar, nc.vector, nc.scalar, nc.vector,
                               nc.scalar, nc.vector, nc.scalar, nc.scalar)[cp_idx]
                        if eng is nc.scalar:
                            nc.scalar.activation(dst, src, ACTF.Copy)
                        else:
                            eng.tensor_copy(dst, src)
                        cp_idx += 1

            # ====== PASS 1b: mm1 k-sweeps, chunk-regions in separate banks ======
            for k in range(ND):
                pk = [psm.tile([P, 2 * NC], F32, name=f"pk{k}_{r}", tag="psm") for r in range(4)]
                for w in range(2):
                    ws = slice(w * NC, (w + 1) * NC)
                    for j in range(NT // 2):
                        for r in range(4):
                            c = w * 4 + r
                            nc.tensor.matmul(
                                pk[r][:, ws],
                                wenc_f8[:, 2 * j : 2 * j + 2, k * P : (k + 1) * P],
                                xt_all[:, 2 * j : 2 * j + 2, c * NC : (c + 1) * NC],
                                start=(j == 0), stop=(j == NT // 2 - 1),
                                perf_mode=DR, skip_group_check=True,
                            )
                    for r in range(4):
                        c = w * 4 + r
                        nc.scalar.activation(
                            encT[:, k, c * NC : (c + 1) * NC], pk[r][:, ws],
                            ACTF.Tanh, bias=bencT[k][:], scale=1.0 / SW_ENC,
                        )

            # ====== PASS 1c: enc_nat / onehot / segment sums / nsq ======
            for i in range(NN):
                nb = i * P
                pe_ = pssm.tile([P, D, 2], F8, tag="pss")
                for k in range(ND):
                    nc.tensor.transpose(
                        pe_[:, k * P : (k + 1) * P, 0:1],
                        encT[:, k, nb : nb + P],
                        ident_f8[:],
                    )
                if i % 2 == 0:
                    nc.vector.tensor_copy(enc_nat[i][:], pe_[:, :, 0])
                else:
                    nc.scalar.activation(enc_nat[i][:], pe_[:, :, 0], ACTF.Copy)

                labi = colsp.tile([P, 1], I32, tag="labi")
                nc.sync.dma_start(
                    labi[:],
                    lab_d[nb : nb + P].rearrange("(p o) -> p o", o=1),
                )
                labf = colsp.tile([P, 1], F32, tag="labf")
                nc.vector.tensor_copy(labf[:], labi[:])
                nc.vector.tensor_scalar(
                    out=onehot[i][:], in0=iotaf[:], scalar1=labf[:],
                    scalar2=None, op0=ALU.is_equal,
                )
                nc.tensor.matmul(
                    seg_ps[:], onehot[i][:], enc_nat[i][:],
                    start=(i == 0), stop=(i == NN - 1),
                    skip_group_check=True,
                )
                nc.tensor.matmul(
                    cnt_ps[:], onehot[i][:], ones_col_f8[:],
                    start=(i == 0), stop=(i == NN - 1),
                    skip_group_check=True,
                )
                nc.scalar.activation(
                    junk_a[:], enc_nat[i][:], ACTF.Square,
                    accum_out=nsq_strip[:, i : i + 1],
                )

            # ---------------- AR#1: segment sums + counts ----------------
            arin = accp.tile([C, D + 1], F32)
            nc.scalar.activation(arin[:, 0:D], seg_ps[:], ACTF.Copy)
            nc.vector.tensor_copy(arin[:, D : D + 1], cnt_ps[:])
            b1in = dp.tile([C, D + 1], F32)
            b1out = dp.tile([C, D + 1], F32)
            nc.sync.dma_start(b1in[:], arin[:])
            nc.gpsimd.collective_compute(
                "AllReduce", ALU.add,
                replica_groups=[list(range(NCORES))],
                ins=[b1in[:].opt()],
                outs=[b1out[:].opt()],
            )
            sums_g = accp.tile([C, D + 1], F32)
            nc.sync.dma_start(sums_g[:], b1out[:])

            # ====== W2 = W_dec @ W_enc prep (runs while AR#1 flies) ======
            wdecT_f8 = wts.tile([P, NT, D], F8)
            for a in range(NT):
                pw = pssm.tile([P, D, 2], F8, tag="pss")
                for j in range(ND):
                    nc.tensor.transpose(
                        pw[:, j * P : (j + 1) * P, 0:1],
                        wdec_f8[:, j, a * P : (a + 1) * P],
                        ident_f8[:],
                    )
                eng = (nc.vector, nc.scalar)[a % 2]
                if eng is nc.scalar:
                    nc.scalar.activation(wdecT_f8[:, a, :], pw[:, :, 0], ACTF.Copy)
                else:
                    eng.tensor_copy(wdecT_f8[:, a, :], pw[:, :, 0])
            w2_f8 = wts.tile([P, ND, D], F8)
            for m in range(ND):
                pw2 = psm.tile([P, D], F32, tag="psm")
                for h in range(2):
                    hs = slice(h * 256, (h + 1) * 256)
                    for a in range(NT // 2):
                        nc.tensor.matmul(
                            pw2[:, hs],
                            wdecT_f8[:, 2 * a : 2 * a + 2, m * P : (m + 1) * P],
                            wenc_f8[:, 2 * a : 2 * a + 2, hs],
                            start=(a == 0), stop=(a == NT // 2 - 1),
                            perf_mode=DR, skip_group_check=True,
                        )
                eng = (nc.vector, nc.scalar)[m % 2]
                if eng is nc.scalar:
                    nc.scalar.activation(
                        w2_f8[:, m, :], pw2[:], ACTF.Copy,
                        scale=SW_W2 / (SW_DEC * SW_ENC),
                    )
                else:
                    eng.tensor_scalar(
                        out=w2_f8[:, m, :], in0=pw2[:],
                        scalar1=SW_W2 / (SW_DEC * SW_ENC),
                        scalar2=None, op0=ALU.mult,
                    )
            pb2 = psm.tile([1, D], F32, tag="psm")
            for h in range(2):
                hs = slice(h * 256, (h + 1) * 256)
                for a in range(NT):
                    nc.tensor.matmul(
                        pb2[:, hs],
                        bdecT_f8[:, a, :],
                        wenc_f8[:, a, hs],
                        start=(a == 0), stop=(a == NT - 1),
                    )
            b2_row = accp.tile([1, D], F32)
            nc.vector.scalar_tensor_tensor(
                out=b2_row[:], in0=pb2[:], scalar=1.0 / (SW_DEC * SW_ENC),
                in1=benc_row[:], op0=ALU.mult, op1=ALU.add,
            )
            b2T = []
            for k in range(ND):
                pt = pssm.tile([P, 1], F32, tag="pss")
                nc.tensor.transpose(
                    pt[:], b2_row[:, k * P : (k + 1) * P], ident_f32[0:1, 0:1]
                )
                bt = wts.tile([P, 1], F32, tag=f"b2T{k}")
                nc.vector.tensor_copy(bt[:], pt[:])
                b2T.append(bt)

            # ====== PASS 2a: decoded + rec pinball + logits/CCE ======
            for i in range(NN):
                nb = i * P
                orow = xrowp.tile([P, T], F32, tag="orow")
                nc.sync.dma_start(orow[:], o_d[nb : nb + P, :])

                pq = [psm.tile([P, D], F32, name=f"pq{i}_{r}", tag="psm") for r in range(4)]
                ps3 = pssm.tile([P, C], F32, tag="pss")
                for h in range(2):
                    hs = slice(h * 256, (h + 1) * 256)
                    for j in range(ND // 2):
                        for q in range(4):
                            nc.tensor.matmul(
                                pq[q][:, hs],
                                encT[:, 2 * j : 2 * j + 2, nb : nb + P],
                                wdec_f8[:, 2 * j : 2 * j + 2,
                                        q * D + h * 256 : q * D + (h + 1) * 256],
                                start=(j == 0), stop=False,
                                perf_mode=DR, skip_group_check=True,
                            )
                        if h == 0:
                            nc.tensor.matmul(
                                ps3[:],
                                encT[:, 2 * j : 2 * j + 2, nb : nb + P],
                                wcls_f8[:, 2 * j : 2 * j + 2, :],
                                start=(j == 0), stop=False,
                                perf_mode=DR, skip_group_check=True,
                            )
                    for q in range(4):
                        nc.tensor.matmul(
                            pq[q][:, hs],
                            ones_pair[:],
                            bdec_pair[:, :, q * D + h * 256 : q * D + (h + 1) * 256],
                            start=False, stop=True,
                            perf_mode=DR, skip_group_check=True,
                        )
                    if h == 0:
                        nc.tensor.matmul(
                            ps3[:], ones_pair[:], bcls_pair[:],
                            start=False, stop=True, perf_mode=DR,
                            skip_group_check=True,
                        )

                for q in range(4):
                    dt_ = dtlp.tile([P, D], BF16, tag="dt")
                    nc.vector.scalar_tensor_tensor(
                        out=dt_[:], in0=orow[:, q * D : (q + 1) * D],
                        scalar=-SW_DEC, in1=pq[q][:], op0=ALU.mult, op1=ALU.add,
                    )
                    col = 4 * i + q
                    if q % 2 == 0:
                        nc.scalar.activation(
                            junk_b[:], dt_[:], ACTF.Abs,
                            accum_out=rec_strip[:, col : col + 1],
                        )
                    else:
                        nc.vector.tensor_reduce(
                            rec_strip[:, col : col + 1], dt_[:], AX.X, ALU.add,
                            apply_absolute_value=True,
                        )

                # logits = ps3/SW_CLS; softmax + one-hot CCE
                nmx32 = colsp.tile([P, 1], F32, tag="nmx32")
                nc.vector.tensor_reduce(nmx32[:], ps3[:], AX.X, ALU.max, negate=True)
                nmx = colsp.tile([P, 1], F32, tag="nmx")
                nc.vector.tensor_scalar(
                    out=nmx[:], in0=nmx32[:], scalar1=1.0 / SW_CLS,
                    scalar2=None, op0=ALU.mult,
                )
                expt = smallp.tile([P, C], BF16, tag="expt")
                sume = colsp.tile([P, 1], F32, tag="sume")
                nc.scalar.activation(
                    expt[:], ps3[:], ACTF.Exp, bias=nmx[:], scale=1.0 / SW_CLS,
                    accum_out=sume[:],
                )
                elab = colsp.tile([P, 1], F32, tag="elab")
                nc.vector.scalar_tensor_tensor(
                    out=junk_c[:], in0=expt[:], scalar=0.0, in1=onehot[i][:],
                    op0=ALU.bypass, op1=ALU.mult, accum_out=elab[:],
                )
                rcs = colsp.tile([P, 1], F32, tag="rcs")
                nc.vector.reciprocal(rcs[:], sume[:])
                plab = colsp.tile([P, 1], F32, tag="plab")
                nc.vector.tensor_tensor(plab[:], elab[:], rcs[:], ALU.mult)
                nc.vector.tensor_scalar(
                    out=cce_strip[:, i : i + 1], in0=plab[:],
                    scalar1=-(C1 - C2), scalar2=C1, op0=ALU.mult, op1=ALU.add,
                )

            # ====== PASS 2b: rec_latents (latT) in i-groups of 4 ======
            for g in range(NN // 4):
                gs = g * 4 * P
                pm = [psm.tile([P, 4 * P], F32, name=f"pm{g}_{r}", tag="psm") for r in range(4)]
                for m in range(ND):
                    for j in range(ND // 2):
                        for t in range(4):
                            nc.tensor.matmul(
                                pm[t][:, m * P : (m + 1) * P],
                                w2_f8[:, 2 * j : 2 * j + 2, m * P : (m + 1) * P],
                                encT[:, 2 * j : 2 * j + 2,
                                     gs + t * P : gs + (t + 1) * P],
                                start=(j == 0), stop=(j == ND // 2 - 1),
                                perf_mode=DR, skip_group_check=True,
                            )
                for t in range(4):
                    lt = ltp.tile([P, 4 * P], BF16, tag="lt")
                    for m in range(ND):
                        nc.scalar.activation(
                            lt[:, m * P : (m + 1) * P],
                            pm[t][:, m * P : (m + 1) * P], ACTF.Tanh,
                            bias=b2T[m][:], scale=1.0 / SW_W2,
                        )
                    ld = dtlp.tile([P, 4 * P], BF16, tag="ld")
                    nc.vector.tensor_tensor(
                        ld[:], lt[:], encT[:, :, gs + t * P : gs + (t + 1) * P],
                        ALU.subtract,
                    )
                    nc.vector.tensor_reduce(
                        lat_strip[:, 4 * g + t : 4 * g + t + 1], ld[:], AX.X,
                        ALU.add, apply_absolute_value=True,
                    )

            # ====== PASS 3a: means / meansT (needs AR#1) ======
            counts_g = accp.tile([C, 1], F32)
            nc.vector.tensor_scalar(
                out=counts_g[:], in0=sums_g[:, D : D + 1], scalar1=1.0,
                scalar2=None, op0=ALU.max,
            )
            crcp = accp.tile([C, 1], F32)
            nc.vector.reciprocal(crcp[:], counts_g[:])
            means = accp.tile([C, D], F32)
            nc.vector.tensor_scalar(
                out=means[:], in0=sums_g[:, 0:D], scalar1=crcp[:],
                scalar2=None, op0=ALU.mult,
            )
            msq_col = accp.tile([C, 1], F32)
            jm = junkp.tile([C, D], BF16, tag="junk_m")
            nc.scalar.activation(jm[:], means[:], ACTF.Square, accum_out=msq_col[:])

            meansT_f8 = accp.tile([P, ND, C], F8)
            for k in range(ND):
                pt = pssm.tile([P, C], F32, tag="pss")
                nc.tensor.transpose(
                    pt[:], means[:, k * P : (k + 1) * P], ident_f32[:C, :C]
                )
                nc.vector.tensor_scalar(
                    out=meansT_f8[:, k, :], in0=pt[:], scalar1=SW_MNS,
                    scalar2=None, op0=ALU.mult,
                )
            pmr = pssm.tile([1, C], F32, tag="pss")
            nc.tensor.transpose(pmr[:], msq_col[:], ident_f32[:C, :C])
            msq_row = accp.tile([1, C], F32)
            nc.scalar.activation(msq_row[:], pmr[:], ACTF.Copy)
            pmb = pssm.tile([P, C], F32, tag="pss")
            nc.tensor.matmul(pmb[:], ones_k1f[:], msq_row[:], start=True, stop=True)
            msq_b = accp.tile([P, C], F32)
            nc.scalar.activation(msq_b[:], pmb[:], ACTF.Copy)

            # ---------------- scalar partials -> AR#2 ----------------
            pack3 = accp.tile([P, 3], F32)
            nc.vector.tensor_reduce(pack3[:, 0:1], rec_strip[:], AX.X, ALU.add)
            nc.vector.tensor_reduce(pack3[:, 1:2], lat_strip[:], AX.X, ALU.add)
            nc.vector.tensor_reduce(pack3[:, 2:3], cce_strip[:], AX.X, ALU.add)
            scps = pssm.tile([1, 3], F32, tag="pss")
            nc.tensor.matmul(scps[:], ones_col[:], pack3[:], start=True, stop=True)
            sc_sb = accp.tile([1, 3], F32)
            nc.scalar.activation(sc_sb[:], scps[:], ACTF.Copy)
            b2in = dp.tile([1, 3], F32)
            b2out = dp.tile([1, 3], F32)
            nc.sync.dma_start(b2in[:], sc_sb[:])
            nc.gpsimd.collective_compute(
                "AllReduce", ALU.add,
                replica_groups=[list(range(NCORES))],
                ins=[b2in[:].opt()],
                outs=[b2out[:].opt()],
            )
            sc_g = accp.tile([1, 3], F32)
            nc.sync.dma_start(sc_g[:], b2out[:])

            # ====== PASS 3b: wgss quadratic terms (overlaps AR#2) ======
            for i in range(NN):
                nb = i * P
                eps_ = pssm.tile([P, C], F32, tag="pss")
                for j in range(ND // 2):
                    nc.tensor.matmul(
                        eps_[:],
                        encT[:, 2 * j : 2 * j + 2, nb : nb + P],
                        meansT_f8[:, 2 * j : 2 * j + 2, :],
                        start=(j == 0), stop=(j == ND // 2 - 1),
                        perf_mode=DR, skip_group_check=True,
                    )
                q_ = smallp.tile([P, C], F32, tag="q")
                nc.vector.scalar_tensor_tensor(
                    out=q_[:], in0=eps_[:], scalar=-2.0 / SW_MNS, in1=msq_b[:],
                    op0=ALU.mult, op1=ALU.add,
                )
                jq = smallp.tile([P, C], BF16, tag="jq")
                nc.vector.scalar_tensor_tensor(
                    out=jq[:], in0=q_[:], scalar=0.0, in1=onehot[i][:],
                    op0=ALU.bypass, op1=ALU.mult,
                    accum_out=gq_strip[:, i : i + 1],
                )

            # ---------------- final combine (needs AR#2) ----------------
            coef = accp.tile([1, 3], F32)
            nc.any.memset(coef[:, 0:1], 0.9 / (n_global * T * SW_DEC))
            nc.any.memset(coef[:, 1:2], 0.9 / (n_global * D))
            nc.any.memset(coef[:, 2:3], 1.0 / n_global)
            sprod = accp.tile([1, 3], F32)
            nc.vector.tensor_tensor(sprod[:], sc_g[:], coef[:], ALU.mult)
            stot = accp.tile([1, 1], F32)
            nc.vector.tensor_reduce(stot[:], sprod[:], AX.X, ALU.add)
            psS = pssm.tile([P, 1], F32, tag="pss")
            nc.tensor.matmul(psS[:], ones_k1f[:], stot[:], start=True, stop=True)
            s_col = accp.tile([P, 1], F32)
            nc.scalar.activation(s_col[:], psS[:], ACTF.Copy)

            for i in range(NN):
                t2 = colsp.tile([P, 1], F32, tag="t2")
                nc.vector.tensor_tensor(
                    t2[:], gq_strip[:, i : i + 1], nsq_strip[:, i : i + 1], ALU.add
                )
                oc = colsp.tile([P, 1], F32, tag="oc")
                nc.vector.scalar_tensor_tensor(
                    out=oc[:], in0=t2[:], scalar=1.0 / D, in1=s_col[:],
                    op0=ALU.mult, op1=ALU.add,
                )
                nc.sync.dma_start(
                    out_d[i * P : (i + 1) * P].rearrange("(p o) -> p o", o=1), oc[:]
                )

    nc.compile()
    return nc


_CACHE = {}


def _get_nc():
    if "nc" not in _CACHE:
        _CACHE["nc"] = build()
    return _CACHE["nc"]


def kernel(**inputs):
    nc = _get_nc()
    nl = N_GLOBAL // NCORES
    shard_names = ["x", "output", "cat_labels", "labels"]
    full_names = ["W_enc", "b_enc", "W_dec", "b_dec", "W_cls", "b_cls"]
    in_maps = []
    for i in range(NCORES):
        m = {}
        for k in shard_names:
            m[k] = np.ascontiguousarray(inputs[k][i * nl : (i + 1) * nl])
        for k in full_names:
            m[k] = np.ascontiguousarray(inputs[k])
        in_maps.append(m)
    res = run_bass_kernel_spmd(nc, in_maps, list(range(NCORES))).results
    return np.concatenate([res[i]["out"] for i in range(NCORES)]).astype(np.float32)


# revision 19
# speedup vs baseline: 1.6920x; 1.6920x over previous
"""Trainium2 Bass kernel for nn_Custom_trainer_79242146611896.

Data-parallel over N=16384 samples on 8 NeuronCores (2048/core).
Per-class segment sums ([C,D]+counts) AllReduce'd early (overlapped with
the decode/reconstruction pass); the 3 scalar loss partials AllReduce'd
late (overlapped with the wgss tail). Weights replicated.

Speed strategy:
  * big matmuls in fp8e4m3 DoubleRow (K=256 per instruction, 0.5
    cycles/row); weights pre-scaled into fp8 normal range and descaled
    in the consuming activation.
  * DR matmuls are ordered for stationary-weight reuse (Ldweights is
    the hidden cost of DR): mm1 runs as k-sweeps over all sample
    chunks per (k,j) stationary; mm2 shares its encT stationaries with
    mm3; rec_latents runs over i-groups of 4 tiles per W2 stationary.
  * rec_latents = tanh(decoded @ W_enc) reuses W2 = W_dec @ W_enc.
  * decoded in natural [n,T] layout; rec pinball = fused
    scalar_tensor_tensor + abs-reduce, split DVE/ACT.
  * CCE exploits cat_labels == one_hot(labels).
  * gpsimd queue carries only weight casts + the two collectives so
    AllReduces never block compute.
"""

import numpy as np

import concourse.bass as bass
import concourse.mybir as mybir
import concourse.tile as tile
from concourse import bacc
from concourse.bass_utils import run_bass_kernel_spmd
from concourse.masks import make_identity

F32 = mybir.dt.float32
F32R = mybir.dt.float32r
BF16 = mybir.dt.bfloat16
F8 = mybir.dt.float8e4
I32 = mybir.dt.int32
AX = mybir.AxisListType
ALU = mybir.AluOpType
ACTF = mybir.ActivationFunctionType
DR = mybir.MatmulPerfMode.DoubleRow

P = 128
NCORES = 8
N_GLOBAL = 16384
T = 2048
D = 512
C = 50
KEPS = 1e-7

SW_ENC = 64.0
SW_DEC = 32.0
SW_CLS = 32.0
SW_W2 = 64.0
SW_MNS = 32.0

C1 = -float(np.log(KEPS))
C2 = -float(np.log1p(-KEPS))


def build(nl=N_GLOBAL // NCORES, n_global=None):
    n_global = n_global or NCORES * nl
    NT = T // P            # 16 K-tiles along T
    ND = D // P            # 4 K-tiles along D
    NN = nl // P           # 16 n-tiles per core
    NC = 256               # transpose chunk (samples)
    NCH = nl // NC         # 8 chunks
    NSUB = NC // P         # 2 n-subtiles per chunk

    nc = bacc.Bacc("TRN2", target_bir_lowering=False, debug=False, num_devices=NCORES)

    x_d = nc.dram_tensor("x", [nl, T], F32, kind="ExternalInput")
    o_d = nc.dram_tensor("output", [nl, T], F32, kind="ExternalInput")
    cl_d = nc.dram_tensor("cat_labels", [nl, C], F32, kind="ExternalInput")
    lab_d = nc.dram_tensor("labels", [nl], I32, kind="ExternalInput")
    wenc_d = nc.dram_tensor("W_enc", [T, D], F32, kind="ExternalInput")
    benc_d = nc.dram_tensor("b_enc", [D], F32, kind="ExternalInput")
    wdec_d = nc.dram_tensor("W_dec", [D, T], F32, kind="ExternalInput")
    bdec_d = nc.dram_tensor("b_dec", [T], F32, kind="ExternalInput")
    wcls_d = nc.dram_tensor("W_cls", [D, C], F32, kind="ExternalInput")
    bcls_d = nc.dram_tensor("b_cls", [C], F32, kind="ExternalInput")
    out_d = nc.dram_tensor("out", [nl], F32, kind="ExternalOutput")

    from contextlib import ExitStack

    with tile.TileContext(nc) as tc:
        with ExitStack() as ctx:
            ent = ctx.enter_context
            constp = ent(tc.tile_pool(name="const", bufs=1))
            wstg = ent(tc.tile_pool(name="wstg", bufs=1))
            wts = ent(tc.tile_pool(name="wts", bufs=1))
            encp = ent(tc.tile_pool(name="enc", bufs=1))
            accp = ent(tc.tile_pool(name="acc", bufs=1))
            xrowp = ent(tc.tile_pool(name="xrow", bufs=2))
            dtlp = ent(tc.tile_pool(name="dtl", bufs=3))
            ltp = ent(tc.tile_pool(name="lt", bufs=2))
            smallp = ent(tc.tile_pool(name="small", bufs=3))
            colsp = ent(tc.tile_pool(name="cols", bufs=4))
            junkp = ent(tc.tile_pool(name="junk", bufs=1))
            psm = ent(tc.tile_pool(name="psm", bufs=4, space="PSUM"))
            pssm = ent(tc.tile_pool(name="pssm", bufs=2, space="PSUM"))
            psper = ent(tc.tile_pool(name="psper", bufs=1, space="PSUM"))
            dp = ent(tc.tile_pool(name="dram", bufs=1, space="DRAM"))

            # ---------------- constants ----------------
            ident_f32 = constp.tile([P, P], F32)
            make_identity(nc, ident_f32)
            ident_f8 = constp.tile([P, P], F8)
            nc.vector.tensor_copy(ident_f8[:], ident_f32[:])

            ones_col = constp.tile([P, 1], F32)
            nc.any.memset(ones_col[:], 1.0)
            ones_col_f8 = constp.tile([P, 1], F8)
            nc.any.memset(ones_col_f8[:], 1.0)
            ones_k1f = constp.tile([1, P], F32)
            nc.any.memset(ones_k1f[:], 1.0)
            ones_pair = constp.tile([1, 2, P], F8)
            nc.any.memset(ones_pair[:, 0:1, :], 1.0)
            nc.any.memset(ones_pair[:, 1:2, :], 0.0)

            iot = constp.tile([P, C], I32)
            nc.gpsimd.iota(iot[:], [[1, C]], channel_multiplier=0)
            iotaf = constp.tile([P, C], F32)
            nc.vector.tensor_copy(iotaf[:], iot[:])

            # ---------------- persistent activations ----------------
            xt_all = encp.tile([P, NT, nl], F8)   # [p, a, n] = x[n, a*128+p]
            encT = encp.tile([P, ND, nl], F8)     # [p, k, n] = enc[n, k*128+p]
            enc_nat = [
                encp.tile([P, D], F8, name=f"encnat{i2}", tag=f"encnat{i2}")
                for i2 in range(NN)
            ]
            onehot = [
                accp.tile([P, C], F8, name=f"oh{i}", tag=f"oh{i}") for i in range(NN)
            ]
            nsq_strip = accp.tile([P, NN], F32)
            rec_strip = accp.tile([P, 4 * NN], F32)
            lat_strip = accp.tile([P, 4 * ND], F32)
            cce_strip = accp.tile([P, NN], F32)
            gq_strip = accp.tile([P, NN], F32)
            junk_a = junkp.tile([P, D], BF16, tag="junk_a")
            junk_b = junkp.tile([P, D], BF16, tag="junk_b")
            junk_c = junkp.tile([P, C], BF16, tag="junk_c")

            seg_ps = psper.tile([C, D], F32)
            cnt_ps = psper.tile([C, 1], F32, tag="cnt")

            # ====== PASS 1a: stream x, PE-transpose into xt_all (fp8) ======
            for c in range(NCH):
                base = c * NC
                xr = []
                for s in range(NSUB):
                    r_ = xrowp.tile([P, T], F32, tag="xrow")
                    nc.sync.dma_start(
                        r_[:], x_d[base + s * P : base + (s + 1) * P, :]
                    )
                    xr.append(r_)
                cp_idx = 0
                for s in range(NSUB):
                    for a4 in range(NT // 4):
                        px = psm.tile([P, 4 * P], F32, tag="psm")
                        for da in range(4):
                            a = 4 * a4 + da
                            nc.tensor.transpose(
                                px[:, da * P : (da + 1) * P],
                                xr[s][:, a * P : (a + 1) * P],
                                ident_f32[:],
                            )
                        dst = xt_all[:, 4 * a4 : 4 * a4 + 4,
                                     base + s * P : base + (s + 1) * P]
                        src = px[:].rearrange("p (a n) -> p a n", a=4)
                        eng = (nc.scalar, nc.vector, nc.scalar, nc.vector,
                               nc.scalar, nc.vector, nc.scalar, nc.scalar)[cp_idx]
                        if eng is nc.scalar:
                            nc.scalar.activation(dst, src, ACTF.Copy)
                        else:
                            eng.tensor_copy(dst, src)
                        cp_idx += 1

            # ---------------- weights: DMA + fp8 casts ----------------
            wenc_st = wstg.tile([P, NT, D], F32, tag="wstg")
            nc.sync.dma_start(
                wenc_st[:], wenc_d.ap().rearrange("(a p) d -> p a d", p=P)
            )
            wenc_f8 = wts.tile([P, NT, D], F8)
            for q in range(4):
                sl = (slice(None), slice(4 * q, 4 * q + 4), slice(None))
                if q % 2 == 0:
                    nc.scalar.activation(wenc_f8[sl], wenc_st[sl], ACTF.Copy,
                                         scale=SW_ENC)
                else:
                    nc.vector.tensor_scalar(
                        out=wenc_f8[sl], in0=wenc_st[sl], scalar1=SW_ENC,
                        scalar2=None, op0=ALU.mult,
                    )
            wdec_f8 = wts.tile([P, ND, T], F8)
            wdec_r = wdec_d.ap().rearrange("(j p) t -> j p t", p=P)
            for q in range(4):
                wdec_st = wstg.tile([P, T], F32, tag="wstg_dec")
                nc.sync.dma_start(wdec_st[:], wdec_r[q])
                if q % 2 == 0:
                    nc.scalar.activation(wdec_f8[:, q, :], wdec_st[:], ACTF.Copy,
                                         scale=SW_DEC)
                else:
                    nc.vector.tensor_scalar(
                        out=wdec_f8[:, q, :], in0=wdec_st[:], scalar1=SW_DEC,
                        scalar2=None, op0=ALU.mult,
                    )
            wcls_st = wstg.tile([P, ND, C], F32, tag="wstg_cls")
            nc.sync.dma_start(
                wcls_st[:], wcls_d.ap().rearrange("(j p) c -> p j c", p=P)
            )
            wcls_f8 = wts.tile([P, ND, C], F8)
            nc.vector.tensor_scalar(
                out=wcls_f8[:], in0=wcls_st[:], scalar1=SW_CLS,
                scalar2=None, op0=ALU.mult,
            )

            benc_r = benc_d.ap().rearrange("(k p) -> k p", p=P)
            bencT = []
            for k in range(ND):
                b_ = wts.tile([P, 1], F32, tag=f"bencT{k}")
                nc.sync.dma_start(b_[:], benc_r[k].rearrange("(p o) -> p o", o=1))
                bencT.append(b_)
            benc_row = wts.tile([1, D], F32)
            nc.sync.dma_start(
                benc_row[:], benc_d.ap().rearrange("(o d) -> o d", o=1)
            )
            bdec_row = wstg.tile([1, T], F32, tag="wstg_bdec")
            nc.sync.dma_start(
                bdec_row[:], bdec_d.ap().rearrange("(o t) -> o t", o=1)
            )
            bdec_pair = wts.tile([1, 2, T], F8)
            nc.any.memset(bdec_pair[:, 1:2, :], 0.0)
            nc.vector.tensor_scalar(
                out=bdec_pair[:, 0, :], in0=bdec_row[:], scalar1=SW_DEC,
                scalar2=None, op0=ALU.mult,
            )
            bdecT_f8 = wts.tile([P, NT, 1], F8)
            bdec_cr = bdec_d.ap().rearrange("(a p) -> a p", p=P)
            for a in range(NT):
                bc = colsp.tile([P, 1], F32, tag="bdec_cst")
                nc.sync.dma_start(bc[:], bdec_cr[a].rearrange("(p o) -> p o", o=1))
                nc.vector.tensor_scalar(
                    out=bdecT_f8[:, a, :], in0=bc[:], scalar1=SW_DEC,
                    scalar2=None, op0=ALU.mult,
                )
            bcls_row = wstg.tile([1, C], F32, tag="wstg_bcls")
            nc.sync.dma_start(
                bcls_row[:], bcls_d.ap().rearrange("(o c) -> o c", o=1)
            )
            bcls_pair = wts.tile([1, 2, C], F8)
            nc.any.memset(bcls_pair[:, 1:2, :], 0.0)
            nc.vector.tensor_scalar(
                out=bcls_pair[:, 0, :], in0=bcls_row[:], scalar1=SW_CLS,
                scalar2=None, op0=ALU.mult,
            )

            # ====== PASS 1b: mm1 k-sweeps, chunk-regions in separate banks ======
            for k in range(ND):
                pk = [psm.tile([P, 2 * NC], F32, name=f"pk{k}_{r}", tag="psm") for r in range(4)]
                for w in range(2):
                    ws = slice(w * NC, (w + 1) * NC)
                    for j in range(NT // 2):
                        for r in range(4):
                            c = w * 4 + r
                            nc.tensor.matmul(
                                pk[r][:, ws],
                                wenc_f8[:, 2 * j : 2 * j + 2, k * P : (k + 1) * P],
                                xt_all[:, 2 * j : 2 * j + 2, c * NC : (c + 1) * NC],
                                start=(j == 0), stop=(j == NT // 2 - 1),
                                perf_mode=DR, skip_group_check=True,
                            )
                    for r in range(4):
                        c = w * 4 + r
                        nc.scalar.activation(
                            encT[:, k, c * NC : (c + 1) * NC], pk[r][:, ws],
                            ACTF.Tanh, bias=bencT[k][:], scale=1.0 / SW_ENC,
                        )

            # ====== PASS 1c: enc_nat / onehot / segment sums / nsq ======
            for i in range(NN):
                nb = i * P
                pe_ = pssm.tile([P, D, 2], F8, tag="pss")
                for k in range(ND):
                    nc.tensor.transpose(
                        pe_[:, k * P : (k + 1) * P, 0:1],
                        encT[:, k, nb : nb + P],
                        ident_f8[:],
                    )
                if i % 2 == 0:
                    nc.vector.tensor_copy(enc_nat[i][:], pe_[:, :, 0])
                else:
                    nc.scalar.activation(enc_nat[i][:], pe_[:, :, 0], ACTF.Copy)

                labi = colsp.tile([P, 1], I32, tag="labi")
                nc.sync.dma_start(
                    labi[:],
                    lab_d[nb : nb + P].rearrange("(p o) -> p o", o=1),
                )
                labf = colsp.tile([P, 1], F32, tag="labf")
                nc.vector.tensor_copy(labf[:], labi[:])
                nc.vector.tensor_scalar(
                    out=onehot[i][:], in0=iotaf[:], scalar1=labf[:],
                    scalar2=None, op0=ALU.is_equal,
                )
                nc.tensor.matmul(
                    seg_ps[:], onehot[i][:], enc_nat[i][:],
                    start=(i == 0), stop=(i == NN - 1),
                    skip_group_check=True,
                )
                nc.tensor.matmul(
                    cnt_ps[:], onehot[i][:], ones_col_f8[:],
                    start=(i == 0), stop=(i == NN - 1),
                    skip_group_check=True,
                )
                nc.scalar.activation(
                    junk_a[:], enc_nat[i][:], ACTF.Square,
                    accum_out=nsq_strip[:, i : i + 1],
                )

            # ---------------- AR#1: segment sums + counts ----------------
            arin = accp.tile([C, D + 1], F32)
            nc.scalar.activation(arin[:, 0:D], seg_ps[:], ACTF.Copy)
            nc.vector.tensor_copy(arin[:, D : D + 1], cnt_ps[:])
            b1in = dp.tile([C, D + 1], F32)
            b1out = dp.tile([C, D + 1], F32)
            nc.sync.dma_start(b1in[:], arin[:])
            nc.gpsimd.collective_compute(
                "AllReduce", ALU.add,
                replica_groups=[list(range(NCORES))],
                ins=[b1in[:].opt()],
                outs=[b1out[:].opt()],
            )
            sums_g = accp.tile([C, D + 1], F32)
            nc.sync.dma_start(sums_g[:], b1out[:])

            # ====== W2 = W_dec @ W_enc prep (runs while AR#1 flies) ======
            wdecT_f8 = wts.tile([P, NT, D], F8)
            for a in range(NT):
                pw = pssm.tile([P, D, 2], F8, tag="pss")
                for j in range(ND):
                    nc.tensor.transpose(
                        pw[:, j * P : (j + 1) * P, 0:1],
                        wdec_f8[:, j, a * P : (a + 1) * P],
                        ident_f8[:],
                    )
                eng = (nc.vector, nc.scalar)[a % 2]
                if eng is nc.scalar:
                    nc.scalar.activation(wdecT_f8[:, a, :], pw[:, :, 0], ACTF.Copy)
                else:
                    eng.tensor_copy(wdecT_f8[:, a, :], pw[:, :, 0])
            w2_f8 = wts.tile([P, ND, D], F8)
            for m in range(ND):
                pw2 = psm.tile([P, D], F32, tag="psm")
                for h in range(2):
                    hs = slice(h * 256, (h + 1) * 256)
                    for a in range(NT // 2):
                        nc.tensor.matmul(
                            pw2[:, hs],
                            wdecT_f8[:, 2 * a : 2 * a + 2, m * P : (m + 1) * P],
                            wenc_f8[:, 2 * a : 2 * a + 2, hs],
                            start=(a == 0), stop=(a == NT // 2 - 1),
                            perf_mode=DR, skip_group_check=True,
                        )
                eng = (nc.vector, nc.scalar)[m % 2]
                if eng is nc.scalar:
                    nc.scalar.activation(
                        w2_f8[:, m, :], pw2[:], ACTF.Copy,
                        scale=SW_W2 / (SW_DEC * SW_ENC),
                    )
                else:
                    eng.tensor_scalar(
                        out=w2_f8[:, m, :], in0=pw2[:],
                        scalar1=SW_W2 / (SW_DEC * SW_ENC),
                        scalar2=None, op0=ALU.mult,
                    )
            pb2 = psm.tile([1, D], F32, tag="psm")
            for h in range(2):
                hs = slice(h * 256, (h + 1) * 256)
                for a in range(NT):
                    nc.tensor.matmul(
                        pb2[:, hs],
                        bdecT_f8[:, a, :],
                        wenc_f8[:, a, hs],
                        start=(a == 0), stop=(a == NT - 1),
                    )
            b2_row = accp.tile([1, D], F32)
            nc.vector.scalar_tensor_tensor(
                out=b2_row[:], in0=pb2[:], scalar=1.0 / (SW_DEC * SW_ENC),
                in1=benc_row[:], op0=ALU.mult, op1=ALU.add,
            )
            b2T = []
            for k in range(ND):
                pt = pssm.tile([P, 1], F32, tag="pss")
                nc.tensor.transpose(
                    pt[:], b2_row[:, k * P : (k + 1) * P], ident_f32[0:1, 0:1]
                )
                bt = wts.tile([P, 1], F32, tag=f"b2T{k}")
                nc.vector.tensor_copy(bt[:], pt[:])
                b2T.append(bt)

            # ====== PASS 2a: decoded + rec pinball + logits/CCE ======
            for i in range(NN):
                nb = i * P
                orow = xrowp.tile([P, T], F32, tag="orow")
                nc.sync.dma_start(orow[:], o_d[nb : nb + P, :])

                pq = [psm.tile([P, D], F32, name=f"pq{i}_{r}", tag="psm") for r in range(4)]
                ps3 = pssm.tile([P, C], F32, tag="pss")
                for h in range(2):
                    hs = slice(h * 256, (h + 1) * 256)
                    for j in range(ND // 2):
                        for q in range(4):
                            nc.tensor.matmul(
                                pq[q][:, hs],
                                encT[:, 2 * j : 2 * j + 2, nb : nb + P],
                                wdec_f8[:, 2 * j : 2 * j + 2,
                                        q * D + h * 256 : q * D + (h + 1) * 256],
                                start=(j == 0), stop=False,
                                perf_mode=DR, skip_group_check=True,
                            )
                        if h == 0:
                            nc.tensor.matmul(
                                ps3[:],
                                encT[:, 2 * j : 2 * j + 2, nb : nb + P],
                                wcls_f8[:, 2 * j : 2 * j + 2, :],
                                start=(j == 0), stop=False,
                                perf_mode=DR, skip_group_check=True,
                            )
                    for q in range(4):
                        nc.tensor.matmul(
                            pq[q][:, hs],
                            ones_pair[:],
                            bdec_pair[:, :, q * D + h * 256 : q * D + (h + 1) * 256],
                            start=False, stop=True,
                            perf_mode=DR, skip_group_check=True,
                        )
                    if h == 0:
                        nc.tensor.matmul(
                            ps3[:], ones_pair[:], bcls_pair[:],
                            start=False, stop=True, perf_mode=DR,
                            skip_group_check=True,
                        )

                for q in range(4):
                    dt_ = dtlp.tile([P, D], BF16, tag="dt")
                    nc.vector.scalar_tensor_tensor(
                        out=dt_[:], in0=orow[:, q * D : (q + 1) * D],
                        scalar=-SW_DEC, in1=pq[q][:], op0=ALU.mult, op1=ALU.add,
                    )
                    col = 4 * i + q
                    if q % 2 == 0:
                        nc.scalar.activation(
                            junk_b[:], dt_[:], ACTF.Abs,
                            accum_out=rec_strip[:, col : col + 1],
                        )
                    else:
                        nc.vector.tensor_reduce(
                            rec_strip[:, col : col + 1], dt_[:], AX.X, ALU.add,
                            apply_absolute_value=True,
                        )

                # logits = ps3/SW_CLS; softmax + one-hot CCE
                nmx32 = colsp.tile([P, 1], F32, tag="nmx32")
                nc.vector.tensor_reduce(nmx32[:], ps3[:], AX.X, ALU.max, negate=True)
                nmx = colsp.tile([P, 1], F32, tag="nmx")
                nc.vector.tensor_scalar(
                    out=nmx[:], in0=nmx32[:], scalar1=1.0 / SW_CLS,
                    scalar2=None, op0=ALU.mult,
                )
                expt = smallp.tile([P, C], BF16, tag="expt")
                sume = colsp.tile([P, 1], F32, tag="sume")
                nc.scalar.activation(
                    expt[:], ps3[:], ACTF.Exp, bias=nmx[:], scale=1.0 / SW_CLS,
                    accum_out=sume[:],
                )
                elab = colsp.tile([P, 1], F32, tag="elab")
                nc.vector.scalar_tensor_tensor(
                    out=junk_c[:], in0=expt[:], scalar=0.0, in1=onehot[i][:],
                    op0=ALU.bypass, op1=ALU.mult, accum_out=elab[:],
                )
                rcs = colsp.tile([P, 1], F32, tag="rcs")
                nc.vector.reciprocal(rcs[:], sume[:])
                plab = colsp.tile([P, 1], F32, tag="plab")
                nc.vector.tensor_tensor(plab[:], elab[:], rcs[:], ALU.mult)
                nc.vector.tensor_scalar(
                    out=cce_strip[:, i : i + 1], in0=plab[:],
                    scalar1=-(C1 - C2), scalar2=C1, op0=ALU.mult, op1=ALU.add,
                )

            # ====== PASS 2b: rec_latents (latT) in i-groups of 4 ======
            for g in range(NN // 4):
                gs = g * 4 * P
                pm = [psm.tile([P, 4 * P], F32, name=f"pm{g}_{r}", tag="psm") for r in range(4)]
                for m in range(ND):
                    for j in range(ND // 2):
                        for t in range(4):
                            nc.tensor.matmul(
                                pm[t][:, m * P : (m + 1) * P],
                                w2_f8[:, 2 * j : 2 * j + 2, m * P : (m + 1) * P],
                                encT[:, 2 * j : 2 * j + 2,
                                     gs + t * P : gs + (t + 1) * P],
                                start=(j == 0), stop=(j == ND // 2 - 1),
                                perf_mode=DR, skip_group_check=True,
                            )
                for t in range(4):
                    lt = ltp.tile([P, 4 * P], BF16, tag="lt")
                    for m in range(ND):
                        nc.scalar.activation(
                            lt[:, m * P : (m + 1) * P],
                            pm[t][:, m * P : (m + 1) * P], ACTF.Tanh,
                            bias=b2T[m][:], scale=1.0 / SW_W2,
                        )
                    ld = dtlp.tile([P, 4 * P], BF16, tag="ld")
                    nc.vector.tensor_tensor(
                        ld[:], lt[:], encT[:, :, gs + t * P : gs + (t + 1) * P],
                        ALU.subtract,
                    )
                    nc.vector.tensor_reduce(
                        lat_strip[:, 4 * g + t : 4 * g + t + 1], ld[:], AX.X,
                        ALU.add, apply_absolute_value=True,
                    )

            # ====== PASS 3a: means / meansT (needs AR#1) ======
            counts_g = accp.tile([C, 1], F32)
            nc.vector.tensor_scalar(
                out=counts_g[:], in0=sums_g[:, D : D + 1], scalar1=1.0,
                scalar2=None, op0=ALU.max,
            )
            crcp = accp.tile([C, 1], F32)
            nc.vector.reciprocal(crcp[:], counts_g[:])
            means = accp.tile([C, D], F32)
            nc.vector.tensor_scalar(
                out=means[:], in0=sums_g[:, 0:D], scalar1=crcp[:],
                scalar2=None, op0=ALU.mult,
            )
            msq_col = accp.tile([C, 1], F32)
            jm = junkp.tile([C, D], BF16, tag="junk_m")
            nc.scalar.activation(jm[:], means[:], ACTF.Square, accum_out=msq_col[:])

            meansT_f8 = accp.tile([P, ND, C], F8)
            for k in range(ND):
                pt = pssm.tile([P, C], F32, tag="pss")
                nc.tensor.transpose(
                    pt[:], means[:, k * P : (k + 1) * P], ident_f32[:C, :C]
                )
                nc.vector.tensor_scalar(
                    out=meansT_f8[:, k, :], in0=pt[:], scalar1=SW_MNS,
                    scalar2=None, op0=ALU.mult,
                )
            pmr = pssm.tile([1, C], F32, tag="pss")
            nc.tensor.transpose(pmr[:], msq_col[:], ident_f32[:C, :C])
            msq_row = accp.tile([1, C], F32)
            nc.scalar.activation(msq_row[:], pmr[:], ACTF.Copy)
            pmb = pssm.tile([P, C], F32, tag="pss")
            nc.tensor.matmul(pmb[:], ones_k1f[:], msq_row[:], start=True, stop=True)
            msq_b = accp.tile([P, C], F32)
            nc.scalar.activation(msq_b[:], pmb[:], ACTF.Copy)

            # ---------------- scalar partials -> AR#2 ----------------
            pack3 = accp.tile([P, 3], F32)
            nc.vector.tensor_reduce(pack3[:, 0:1], rec_strip[:], AX.X, ALU.add)
            nc.vector.tensor_reduce(pack3[:, 1:2], lat_strip[:], AX.X, ALU.add)
            nc.vector.tensor_reduce(pack3[:, 2:3], cce_strip[:], AX.X, ALU.add)
            scps = pssm.tile([1, 3], F32, tag="pss")
            nc.tensor.matmul(scps[:], ones_col[:], pack3[:], start=True, stop=True)
            sc_sb = accp.tile([1, 3], F32)
            nc.scalar.activation(sc_sb[:], scps[:], ACTF.Copy)
            b2in = dp.tile([1, 3], F32)
            b2out = dp.tile([1, 3], F32)
            nc.sync.dma_start(b2in[:], sc_sb[:])
            nc.gpsimd.collective_compute(
                "AllReduce", ALU.add,
                replica_groups=[list(range(NCORES))],
                ins=[b2in[:].opt()],
                outs=[b2out[:].opt()],
            )
            sc_g = accp.tile([1, 3], F32)
            nc.sync.dma_start(sc_g[:], b2out[:])

            # ====== PASS 3b: wgss quadratic terms (overlaps AR#2) ======
            for i in range(NN):
                nb = i * P
                eps_ = pssm.tile([P, C], F32, tag="pss")
                for j in range(ND // 2):
                    nc.tensor.matmul(
                        eps_[:],
                        encT[:, 2 * j : 2 * j + 2, nb : nb + P],
                        meansT_f8[:, 2 * j : 2 * j + 2, :],
                        start=(j == 0), stop=(j == ND // 2 - 1),
                        perf_mode=DR, skip_group_check=True,
                    )
                q_ = smallp.tile([P, C], F32, tag="q")
                nc.vector.scalar_tensor_tensor(
                    out=q_[:], in0=eps_[:], scalar=-2.0 / SW_MNS, in1=msq_b[:],
                    op0=ALU.mult, op1=ALU.add,
                )
                jq = smallp.tile([P, C], BF16, tag="jq")
                nc.vector.scalar_tensor_tensor(
                    out=jq[:], in0=q_[:], scalar=0.0, in1=onehot[i][:],
                    op0=ALU.bypass, op1=ALU.mult,
                    accum_out=gq_strip[:, i : i + 1],
                )

            # ---------------- final combine (needs AR#2) ----------------
            coef = accp.tile([1, 3], F32)
            nc.any.memset(coef[:, 0:1], 0.9 / (n_global * T * SW_DEC))
            nc.any.memset(coef[:, 1:2], 0.9 / (n_global * D))
            nc.any.memset(coef[:, 2:3], 1.0 / n_global)
            sprod = accp.tile([1, 3], F32)
            nc.vector.tensor_tensor(sprod[:], sc_g[:], coef[:], ALU.mult)
            stot = accp.tile([1, 1], F32)
            nc.vector.tensor_reduce(stot[:], sprod[:], AX.X, ALU.add)
            psS = pssm.tile([P, 1], F32, tag="pss")
            nc.tensor.matmul(psS[:], ones_k1f[:], stot[:], start=True, stop=True)
            s_col = accp.tile([P, 1], F32)
            nc.scalar.activation(s_col[:], psS[:], ACTF.Copy)

            for i in range(NN):
                t2 = colsp.tile([P, 1], F32, tag="t2")
                nc.vector.tensor_tensor(
                    t2[:], gq_strip[:, i : i + 1], nsq_strip[:, i : i + 1], ALU.add
                )
                oc = colsp.tile([P, 1], F32, tag="oc")
                nc.vector.scalar_tensor_tensor(
                    out=oc[:], in0=t2[:], scalar=1.0 / D, in1=s_col[:],
                    op0=ALU.mult, op1=ALU.add,
                )
                nc.sync.dma_start(
                    out_d[i * P : (i + 1) * P].rearrange("(p o) -> p o", o=1), oc[:]
                )

    nc.compile()
    return nc


_CACHE = {}


def _get_nc():
    if "nc" not in _CACHE:
        _CACHE["nc"] = build()
    return _CACHE["nc"]


def kernel(**inputs):
    nc = _get_nc()
    nl = N_GLOBAL // NCORES
    shard_names = ["x", "output", "cat_labels", "labels"]
    full_names = ["W_enc", "b_enc", "W_dec", "b_dec", "W_cls", "b_cls"]
    in_maps = []
    for i in range(NCORES):
        m = {}
        for k in shard_names:
            m[k] = np.ascontiguousarray(inputs[k][i * nl : (i + 1) * nl])
        for k in full_names:
            m[k] = np.ascontiguousarray(inputs[k])
        in_maps.append(m)
    res = run_bass_kernel_spmd(nc, in_maps, list(range(NCORES))).results
    return np.concatenate([res[i]["out"] for i in range(NCORES)]).astype(np.float32)
